# revision 55
# baseline (speedup 1.0000x reference)
"""Trainium2 Bass kernel for nn_GatedShortBlock, v5: Winograd-Strassen mm1.

Math (per batch b):
  BCx = x @ w1.T ; Bg, Cg, Xg = split(BCx, 3)
  gated = Bg * Xg
  conv  = causal depthwise conv1d(gated, conv_w, K=4)  (left pad 3)
  out   = (Cg * conv) @ w2.T

Sharding: data-parallel over (batch, seq-half) -> 8 shards of 2048 tokens.

mm1 (x @ w1.T, [6144x2048]@[2048x1024tok] per block) uses one level of
Winograd-Strassen: 2x2 blocking over (out 3072 | K 1024 | tok 512) runs 7
half-size products instead of 8 (12.5% fewer PE cycles, the kernel's
dominant cost). A-side combos are folded into the host-packed weights; the
four B-side (x) combos are built on DVE/GPSIMD in bf16; the quadrant
recombination drains PSUM product banks into SBUF via a bank-lifetime
schedule (P1,P2,P6,P7,P5,P4,P3) that never holds more than ~7 banks.

  S1=A21+A22 S2=S1-A11 S3=A11-A21 S4=A12-S2       (host)
  T1=B12-B11 T2=B22-T1 T3=B22-B12 T4=T2-B21       (DVE/GPSIMD, bf16)
  P1=A11*B11 P2=A12*B21 P3=S4*B22 P4=A22*T4 P5=S1*T1 P6=S2*T2 P7=S3*T3
  C11=P1+P2  U1=P1+P6  U2=U1+P7  U3=U1+P5
  C21=U2-P4  C22=U2+P5 C12=U3+P3

Position order pairs (8+j, 16+j) so Bg/Xg gating and Cg availability line
up with a 3-deep gated ring; conv runs on GPSIMD; R overwrites the Bg
tiles in place of a separate pool. Outputs store as bf16 (host upcasts).
"""

import sys

sys.path.insert(0, "/opt/trn_rl_repo")

import numpy as np
import ml_dtypes
from contextlib import ExitStack

import concourse.bass as bass
import concourse.tile as tile
from concourse import bacc, mybir
from concourse.bass_utils import run_bass_kernel_spmd

F32 = mybir.dt.float32
BF16 = mybir.dt.bfloat16
NPBF16 = ml_dtypes.bfloat16
KS = 4  # conv kernel size
KG = 4  # k-subtiles per weight DMA tile
D = 2048
T = 2048  # tokens per core
TBLK = 1024
HT = 512  # token half within a block (Strassen N split)
HK = 1024  # contraction half (Strassen K split)
HM = 3072  # output-row half (Strassen M split)
CH = 512
NWARM = 10  # PE warmup matmuls issued before the first data-dependent matmul

ND = D // 128  # 16 k-tiles over full K (mm2)
NDH = HK // 128  # 8 k-tiles over half K (mm1 products)
NC = D // 128  # 16 channel tiles
NE2 = HM // 128  # 24 e-tiles per product
NBLK = T // TBLK  # 2
NG = ND // KG  # 4 weight-DMA tiles per mm2 e-tile
NG2 = NDH // KG  # 2 weight-DMA tiles per product e-tile

# product emission order = bank-lifetime schedule (P1,P2,P6,P7,P5,P4,P3)
PORDER = [0, 1, 5, 6, 4, 3, 2]
# position order: 0..7 emit Bg[0..7]+Cg[8..15]; then (8+j,16+j) pairs so
# gating (needs Bg[jt], Xg[jt]) and R (needs Cg[jt], conv) resolve with a
# short gated ring.
POS = list(range(8)) + [e for j in range(8) for e in (8 + j, 16 + j)]


def build_program():
    nc = bacc.Bacc(None)
    xTb = nc.dram_tensor("xTb", [D, T], BF16, kind="ExternalInput")
    # wS tile (p,e,g) at rows ((p*NE2+e)*NG2+g)*128; [q, ks*128+m] =
    # Wp[e*128+m, (g*KG+ks)*128+q]  (Wp = p-th Strassen weight combo)
    wS = nc.dram_tensor("wS", [7 * NE2 * NG2 * 128, KG * 128], BF16, kind="ExternalInput")
    w2P = nc.dram_tensor("w2P", [NC * NG * 128, KG * 128], BF16, kind="ExternalInput")
    cw2 = nc.dram_tensor("cw2", [128, NC * KS], F32, kind="ExternalInput")
    gh2 = nc.dram_tensor("gh2", [128, NC * (KS - 1)], F32, kind="ExternalInput")
    outT = nc.dram_tensor("outT", [D, T], BF16, kind="ExternalOutput")

    with tile.TileContext(nc) as tc, ExitStack() as ctx:
        # weight pools FIRST: LDWEIGHTS SBUF reads get slower the higher the
        # weight tile's SBUF address — keep weights at low addresses.
        wp = ctx.enter_context(tc.tile_pool(name="wp", bufs=20))
        w2p = ctx.enter_context(tc.tile_pool(name="w2p", bufs=8))
        xp = ctx.enter_context(tc.tile_pool(name="xp", bufs=1))
        tcp = ctx.enter_context(tc.tile_pool(name="tcp", bufs=1))
        gwp = ctx.enter_context(tc.tile_pool(name="gwp", bufs=1))
        cgp = ctx.enter_context(tc.tile_pool(name="cgp", bufs=1))
        xgp = ctx.enter_context(tc.tile_pool(name="xgp", bufs=2))
        gtp = ctx.enter_context(tc.tile_pool(name="gtp", bufs=3))
        upo = ctx.enter_context(tc.tile_pool(name="upo", bufs=5))
        scrp = ctx.enter_context(tc.tile_pool(name="scrp", bufs=6))
        stgp = ctx.enter_context(tc.tile_pool(name="stgp", bufs=3))
        smallp = ctx.enter_context(tc.tile_pool(name="smallp", bufs=1))
        psp = ctx.enter_context(tc.tile_pool(name="psp", bufs=8, space="PSUM"))

        cwt = smallp.tile([128, NC * KS], F32, tag="cw", name="cw")
        ghS = smallp.tile([128, NC * (KS - 1)], F32, tag="gh", name="gh")
        ghsb = [
            smallp.tile([128, KS - 1], F32, tag=f"ghc{c}", name=f"ghc{c}")
            for c in range(NC)
        ]

        # PE warmup: the HAM clock gate keeps the PE at 1.2 GHz until it has
        # been busy ~3.4us; burn dummy matmuls while the first DMAs fly.
        jtile = smallp.tile([128, 512], BF16, tag="junk", name="junk")
        nc.gpsimd.memset(jtile[:], 0.0)
        psw = psp.tile([128, 512], F32, tag="ps", name="wup_ps")
        for i in range(NWARM):
            nc.tensor.matmul(psw[:], jtile[:, 0:128], jtile[:], start=True, stop=True)

        def load_wS(p, e, b):
            tiles = []
            base = (p * NE2 + e) * NG2
            for g in range(NG2):
                wt = wp.tile([128, KG * 128], BF16, tag="w1", name=f"wS_{b}_{p}_{e}_{g}")
                # startup: sync alone can't feed x + early weights (~230GB/s
                # demanded); odd-g tiles of the first positions go on scalar
                eng = nc.scalar if (b == 0 and e < 6 and g == 1) else nc.sync
                eng.dma_start(wt[:], wS[(base + g) * 128 : (base + g + 1) * 128, :])
                tiles.append(wt)
            return tiles

        def load_w2(f, b):
            # scalar queue: bypasses the 43MB/block wS stream on sync
            tiles = []
            for g in range(NG):
                wt = w2p.tile([128, KG * 128], BF16, tag="w2", name=f"w2_{b}_{f}_{g}")
                nc.scalar.dma_start(
                    wt[:], w2P[(f * NG + g) * 128 : (f * NG + g + 1) * 128, :]
                )
                tiles.append(wt)
            return tiles

        for b in range(NBLK):
            with nc.named_scope(f"blk{b}"):
                # ---- x loads ----
                # (k, k+8) pairs land together so combo k can start after two
                # tiles; upper half on the idle scalar queue at startup
                xt = [None] * ND
                korder = [k for j in range(8) for k in (j, j + 8)] if b == 0 else range(ND)
                for k in korder:
                    t = xp.tile([128, TBLK], BF16, tag=f"x{k}", name=f"x{k}_{b}")
                    if b == 0 and k == 0:
                        # split so matmul 0 only waits on a half tile
                        nc.sync.dma_start(t[:, 0:HT], xTb[0:128, 0:HT])
                        nc.sync.dma_start(t[:, HT:TBLK], xTb[0:128, HT:TBLK])
                    elif b == 0 and k >= 8:
                        nc.scalar.dma_start(t[:], xTb[k * 128 : (k + 1) * 128, 0:TBLK])
                    else:
                        nc.sync.dma_start(
                            t[:], xTb[k * 128 : (k + 1) * 128, b * TBLK : (b + 1) * TBLK]
                        )
                    xt[k] = t
                    if b == 0 and k == 0:
                        wfirst = load_wS(0, 0, 0)
                        nc.scalar.dma_start(ghS[:], gh2[:, :])
                        nc.scalar.dma_start(cwt[:], cw2[:, :])

                # ---- B-side Strassen combos (bf16), split DVE/GPSIMD ----
                t1 = []
                t2 = []
                t3 = []
                t4 = []

                for k in range(NDH):
                    eng = nc.vector if k % 2 == 0 else nc.gpsimd
                    a = tcp.tile([128, HT], BF16, tag=f"t1_{k}", name=f"t1_{k}_{b}")
                    eng.tensor_sub(a[:], xt[k][:, HT:TBLK], xt[k][:, 0:HT])
                    c = tcp.tile([128, HT], BF16, tag=f"t3_{k}", name=f"t3_{k}_{b}")
                    eng.tensor_sub(c[:], xt[8 + k][:, HT:TBLK], xt[k][:, HT:TBLK])
                    d = tcp.tile([128, HT], BF16, tag=f"t2_{k}", name=f"t2_{k}_{b}")
                    eng.tensor_sub(d[:], xt[8 + k][:, HT:TBLK], a[:])
                    f = tcp.tile([128, HT], BF16, tag=f"t4_{k}", name=f"t4_{k}_{b}")
                    eng.tensor_sub(f[:], d[:], xt[8 + k][:, 0:HT])
                    t1.append(a)
                    t2.append(d)
                    t3.append(c)
                    t4.append(f)

                def moving(p, k):
                    if p == 0:
                        return xt[k][:, 0:HT]
                    if p == 1:
                        return xt[8 + k][:, 0:HT]
                    if p == 2:
                        return xt[8 + k][:, HT:TBLK]
                    if p == 3:
                        return t4[k][:]
                    if p == 4:
                        return t1[k][:]
                    if p == 5:
                        return t2[k][:]
                    return t3[k][:]

                gw = {}
                cg = {}
                gts = {}
                for c in range(NC):
                    gw[c] = gwp.tile([128, TBLK], BF16, tag=f"gw{c}", name=f"gw{c}_{b}")
                    cg[c] = cgp.tile([128, TBLK], BF16, tag=f"cg{c}", name=f"cg{c}_{b}")

                sconv = {}

                def conv_chain(jt):
                    # 4-tap causal conv on gated (DVE; Pool lacks the
                    # scalar-pointer tensor_scalar opcode)
                    gt = gts[jt]
                    s = scrp.tile([128, TBLK], F32, tag="scr", name=f"s0_{b}_{jt}")
                    # tap 0 on ACT (out = in * scale, per-partition scale AP)
                    nc.scalar.activation(
                        s[:],
                        gt[:, 0:TBLK],
                        mybir.ActivationFunctionType.Identity,
                        scale=cwt[:, jt * KS : jt * KS + 1],
                    )
                    for j in range(1, KS):
                        s2 = scrp.tile([128, TBLK], F32, tag="scr", name=f"s{j}_{b}_{jt}")
                        nc.vector.scalar_tensor_tensor(
                            s2[:],
                            gt[:, j : j + TBLK],
                            cwt[:, jt * KS + j : jt * KS + j + 1],
                            s[:],
                            mybir.AluOpType.mult,
                            mybir.AluOpType.add,
                        )
                        s = s2
                    sconv[jt] = s

                def r_mul(jt, eng=None):
                    # R = Cg * conv, overwriting the dead Bg content of gw[jt]
                    s = sconv[jt]
                    for u in range(2):
                        (eng or nc.gpsimd).tensor_mul(
                            gw[jt][:, u * HT : (u + 1) * HT],
                            s[:, u * HT : (u + 1) * HT],
                            cg[jt][:, u * HT : (u + 1) * HT],
                        )

                # ---- mm1: 7 Strassen products + recombination per position ----
                ps2_pre = {}
                for ei, e in enumerate(POS):
                    banks = {}
                    # block-0 head: raw-x products (P1,P2,P3) first so the PE
                    # streams before the B-side combos are ready
                    porder = [0, 1, 2, 5, 6, 4, 3] if (b == 0 and ei < 2) else PORDER
                    for np_, p in enumerate(porder):
                        wts = (
                            wfirst
                            if (b == 0 and e == 0 and p == 0)
                            else load_wS(p, e, b)
                        )
                        bank = psp.tile([128, HT], F32, tag="ps", name=f"pb{b}_{e}_{p}")
                        banks[p] = bank
                        for k in range(NDH):
                            w_ap = wts[k // KG][:, (k % KG) * 128 : (k % KG + 1) * 128]
                            nc.tensor.matmul(
                                bank[:],
                                w_ap,
                                moving(p, k),
                                start=(k == 0),
                                stop=(k == NDH - 1),
                            )
                        if ei == len(POS) - 1 and np_ in (1, 3):
                            # hoist mm2 f=0/1 accumulators onto ring slots
                            # freed by the previous position so mm2 can start
                            # the moment the last products finish
                            fi_ = 0 if np_ == 1 else 1
                            ps2_pre[fi_] = [
                                psp.tile(
                                    [128, CH], F32, tag="ps", name=f"ps2{b}_{fi_}_{u}"
                                )
                                for u in range(2)
                            ]
                    # recombination (bank-lifetime order; copies on ACT,
                    # u2-consumers on GPSIMD, rest on DVE)
                    sU = upo.tile([128, HT], F32, tag="u", name=f"sU{b}_{e}")
                    nc.scalar.copy(sU[:], banks[0][:])
                    d11 = gw[e][:, 0:HT] if e < 16 else cg[e - 16][:, 0:HT]
                    nc.vector.tensor_add(d11, sU[:], banks[1][:])
                    u1 = upo.tile([128, HT], F32, tag="u", name=f"u1{b}_{e}")
                    nc.vector.tensor_add(u1[:], sU[:], banks[5][:])
                    u2 = upo.tile([128, HT], F32, tag="u", name=f"u2{b}_{e}")
                    nc.vector.tensor_add(u2[:], u1[:], banks[6][:])
                    u3 = upo.tile([128, HT], F32, tag="u", name=f"u3{b}_{e}")
                    nc.vector.tensor_add(u3[:], u1[:], banks[4][:])
                    if e < 8:
                        d21 = cg[8 + e][:, 0:HT]
                        d22 = cg[8 + e][:, HT:TBLK]
                        xg_t = None
                    else:
                        xg_t = xgp.tile([128, TBLK], BF16, tag="xg", name=f"xg{b}_{e}")
                        d21 = xg_t[:, 0:HT]
                        d22 = xg_t[:, HT:TBLK]
                    nc.vector.tensor_add(d22, u2[:], banks[4][:])
                    nc.vector.tensor_sub(d21, u2[:], banks[3][:])
                    d12 = gw[e][:, HT:TBLK] if e < 16 else cg[e - 16][:, HT:TBLK]
                    nc.vector.tensor_add(d12, u3[:], banks[2][:])

                    if e >= 8:
                        jt = e - 8
                        gt = gtp.tile(
                            [128, KS - 1 + TBLK], BF16, tag="gt", name=f"gt{b}_{jt}"
                        )
                        if b == 0:
                            nc.gpsimd.tensor_copy(
                                gt[:, 0 : KS - 1],
                                ghS[:, jt * (KS - 1) : (jt + 1) * (KS - 1)],
                            )
                        else:
                            nc.gpsimd.tensor_copy(gt[:, 0 : KS - 1], ghsb[jt][:])
                        nc.gpsimd.tensor_mul(gt[:, KS - 1 :], gw[jt][:], xg_t[:])
                        gts[jt] = gt
                        if b < NBLK - 1:
                            nc.gpsimd.tensor_copy(
                                ghsb[jt][:], gt[:, TBLK : TBLK + KS - 1]
                            )
                        conv_chain(jt)
                    if e >= 16:
                        r_mul(e - 16)
                        r_mul(e - 8)

                # ---- mm2: out = R.T @ w2.T (channel-major) ----
                last = b == NBLK - 1

                def stores(fi, ps2):
                    for u in range(2):
                        st = stgp.tile([128, CH], BF16, tag="stg", name=f"st{b}_{fi}_{u}")
                        nc.scalar.copy(st[:], ps2[u][:])
                        dst = outT[
                            fi * 128 : (fi + 1) * 128,
                            b * TBLK + u * CH : b * TBLK + (u + 1) * CH,
                        ]
                        if last and fi >= NC - 2:
                            h = CH // 2
                            nc.scalar.dma_start(dst[:, 0:h], st[:, 0:h])
                            nc.sync.dma_start(dst[:, h:CH], st[:, h:CH])
                        else:
                            eng = nc.sync if (last and (fi * 2 + u) % 2) else nc.scalar
                            eng.dma_start(dst, st[:])

                # f=0 and f=1 interleaved in R-completion c-order: ~25us of
                # ready matmuls sit ahead of the R[15]-dependent ones, hiding
                # the last position's recombo->gate->conv->R tail
                w2t01 = [load_w2(0, b), load_w2(1, b)]
                corder = [c for j in range(8) for c in (j, 8 + j)]
                for ci, c in enumerate(corder):
                    if b == 0 and ci == len(corder) - 1:
                        # the R[15] chain stalls the PE ~4.4us here — longer
                        # than the 3.4us HAM window. Junk matmuls keep the
                        # clock gate open through the bubble.
                        jb = psp.tile([128, 512], F32, tag="ps", name="jbridge")
                        for _ in range(18):
                            nc.tensor.matmul(
                                jb[:], jtile[:, 0:128], jtile[:], start=True, stop=True
                            )
                    g, cs = divmod(c, KG)
                    for f in range(2):
                        w_ap = w2t01[f][g][:, cs * 128 : (cs + 1) * 128]
                        for u in range(2):
                            nc.tensor.matmul(
                                ps2_pre[f][u][:],
                                w_ap,
                                gw[c][:, u * CH : (u + 1) * CH],
                                start=(ci == 0),
                                stop=(ci == NC - 1),
                            )
                w2next = load_w2(2, b)
                for f in range(2):
                    stores(f, ps2_pre[f])
                for fi in range(2, NC):
                    w2t = w2next
                    if fi + 1 < NC:
                        w2next = load_w2(fi + 1, b)
                    ps2 = [
                        psp.tile([128, CH], F32, tag="ps", name=f"ps2{b}_{fi}_{u}")
                        for u in range(2)
                    ]
                    for ci, c in enumerate(range(NC)):
                        g, cs = divmod(c, KG)
                        w_ap = w2t[g][:, cs * 128 : (cs + 1) * 128]
                        for u in range(2):
                            nc.tensor.matmul(
                                ps2[u][:],
                                w_ap,
                                gw[c][:, u * CH : (u + 1) * CH],
                                start=(ci == 0),
                                stop=(ci == NC - 1),
                            )
                    stores(fi, ps2)

    nc.finalize()
    return nc


def _pack_tiles(w, n_etiles, ng):
    """[n_etiles*128, ng*KG*128] -> packed [(e*ng+g)*128 + q, ks*128+m] =
    w[e*128+m, (g*KG+ks)*128+q], contiguous per [128, 512] tile."""
    return np.ascontiguousarray(
        w.reshape(n_etiles, 128, ng, KG, 128)
        .transpose(0, 2, 4, 3, 1)
        .reshape(n_etiles * ng * 128, KG * 128)
    ).astype(NPBF16)


def shard_inputs(x, w1, w2, conv_w):
    B, S, _ = x.shape
    n_shards = (B * S) // T

    A11 = w1[0:HM, 0:HK]
    A12 = w1[0:HM, HK:]
    A21 = w1[HM:, 0:HK]
    A22 = w1[HM:, HK:]
    S1 = A21 + A22
    S2 = S1 - A11
    S3 = A11 - A21
    S4 = A12 - S2
    wSk = np.concatenate(
        [_pack_tiles(M, NE2, NG2) for M in (A11, A12, S4, A22, S1, S2, S3)], axis=0
    )
    w2Pk = _pack_tiles(w2, NC, NG)
    cw2 = np.ascontiguousarray(
        conv_w[:, 0, :].reshape(NC, 128, KS).transpose(1, 0, 2).reshape(128, NC * KS)
    ).astype(np.float32)

    shards_per_batch = S // T
    in_maps = []
    for s in range(n_shards):
        b, h = divmod(s, shards_per_batch)
        xs = x[b, h * T : (h + 1) * T, :]
        xTs = np.ascontiguousarray(xs.T).astype(NPBF16)
        if h == 0:
            gh2 = np.zeros((128, NC * (KS - 1)), np.float32)
        else:
            xh = x[b, h * T - (KS - 1) : h * T, :].astype(NPBF16).astype(np.float32)
            Bg = xh @ w1[0:D].T
            Xg = xh @ w1[2 * D : 3 * D].T
            ghs = np.ascontiguousarray((Bg * Xg).T).astype(np.float32)  # [D, 3]
            gh2 = np.ascontiguousarray(
                ghs.reshape(NC, 128, KS - 1).transpose(1, 0, 2).reshape(
                    128, NC * (KS - 1)
                )
            )
        in_maps.append({"xTb": xTs, "wS": wSk, "w2P": w2Pk, "cw2": cw2, "gh2": gh2})
    return in_maps


_PROGRAM_CACHE = {}


def run(x, w1, w2, conv_w, trace=False):
    B, S, _ = x.shape
    if "p" not in _PROGRAM_CACHE:
        _PROGRAM_CACHE["p"] = build_program()
    nc = _PROGRAM_CACHE["p"]
    in_maps = shard_inputs(x, w1, w2, conv_w)
    n_shards = len(in_maps)
    res = run_bass_kernel_spmd(nc, in_maps, core_ids=list(range(n_shards)), trace=trace)
    shards_per_batch = S // T
    out = np.empty((B, S, D), np.float32)
    for s in range(n_shards):
        b, h = divmod(s, shards_per_batch)
        out[b, h * T : (h + 1) * T, :] = res.results[s]["outT"].T.astype(np.float32)
    return out, res


def kernel(x, w1, w2, conv_w):
    x = np.asarray(x, np.float32)
    w1 = np.asarray(w1, np.float32)
    w2 = np.asarray(w2, np.float32)
    conv_w = np.asarray(conv_w, np.float32)
    out, _ = run(x, w1, w2, conv_w)
    return out
